# revision 1
# baseline (speedup 1.0000x reference)
"""Sparse (DAG-masked) attention head on 8 Trainium2 NeuronCores.

Reference computation (per batch b of 64):
    K = X_b @ Wk + bk; Q = Y_b @ Wq + bq; V = X_b @ Wv + bv         [T=1024, H=512]
    S = Q @ K^T / sqrt(H); A = softmax(where(dag.T*S == 0, -inf, dag.T*S))
    O = A @ V   (fully-masked rows -> 0)

Strategy: data-parallel over batch (8 batches per core); weights + dag
replicated. All matmuls run in float32r (TF32-like, 1 cycle/row on PE,
~1e-4 relative error). Scores are computed TRANSPOSED (ST[s,t]) so that
softmax-weights PT = dag * exp(ST/sqrt(H)) land directly in the [s, t]
layout needed as the stationary operand of the P @ V matmul -- no
on-chip transposes anywhere. Softmax skips max-subtraction (scores are
~N(0,1); exp can't overflow) and normalizes AFTER the V-matmul using
column sums l obtained from a ones-column matmul.

Host-side prep: X/Y are transposed to [D, T] per batch (the PE contracts
over the partition dim, so projections need d-major activations).
"""

import numpy as np

import concourse.bass as bass
import concourse.mybir as mybir
import concourse.tile as tile
from concourse import bacc
from concourse.bass_utils import run_bass_kernel_spmd

B, T, D, H = 64, 1024, 512, 512
NCORES = 8
BPC = B // NCORES          # batches per core
DC = D // 128              # d chunks (4)
HC = H // 128              # h tiles (4)
TC = T // 128              # t/s tiles (8)
SCALE = 1.0 / float(np.sqrt(H))

f32 = mybir.dt.float32
f32r = mybir.dt.float32r
EXP = mybir.ActivationFunctionType.Exp
IDENT = mybir.ActivationFunctionType.Identity

_CACHED_NC = None


def _build():
    nc = bacc.Bacc("TRN2", target_bir_lowering=False, debug=False,
                   num_devices=NCORES)

    XTd = nc.dram_tensor("XT", [BPC, DC, 128, T], f32r, kind="ExternalInput").ap()
    YTd = nc.dram_tensor("YT", [BPC, DC, 128, T], f32r, kind="ExternalInput").ap()
    DAGd = nc.dram_tensor("dagr", [TC, 128, T], f32, kind="ExternalInput").ap()
    Wkd = nc.dram_tensor("Wkr", [DC, 128, H], f32r, kind="ExternalInput").ap()
    Wqd = nc.dram_tensor("Wqr", [DC, 128, H], f32r, kind="ExternalInput").ap()
    Wvd = nc.dram_tensor("Wvr", [DC, 128, H], f32r, kind="ExternalInput").ap()
    Bkd = nc.dram_tensor("bkt", [128, HC], f32, kind="ExternalInput").ap()
    Bqd = nc.dram_tensor("bqt", [128, HC], f32, kind="ExternalInput").ap()
    Bvd = nc.dram_tensor("bvb", [128, H], f32, kind="ExternalInput").ap()
    ONESd = nc.dram_tensor("ones", [128, 2], f32r, kind="ExternalInput").ap()
    Od = nc.dram_tensor("O", [BPC, T, H], f32, kind="ExternalOutput").ap()

    with tile.TileContext(nc) as tc:
        with (
            tc.tile_pool(name="const", bufs=1) as const,
            tc.tile_pool(name="data", bufs=1) as data,
            tc.tile_pool(name="small", bufs=3) as small,
            tc.tile_pool(name="psum", bufs=2, space="PSUM") as psum,
        ):
            # ---- resident tensors ----
            wk = const.tile([128, DC, H], f32r, tag="wk")
            wq = const.tile([128, DC, H], f32r, tag="wq")
            wv = const.tile([128, DC, H], f32r, tag="wv")
            for c in range(DC):
                nc.sync.dma_start(out=wk[:, c], in_=Wkd[c])
                nc.sync.dma_start(out=wq[:, c], in_=Wqd[c])
                nc.sync.dma_start(out=wv[:, c], in_=Wvd[c])
            dag = const.tile([128, TC, T], f32, tag="dag")
            for i in range(TC):
                nc.sync.dma_start(out=dag[:, i], in_=DAGd[i])
            bkt = const.tile([128, HC], f32, tag="bkt")
            bqt = const.tile([128, HC], f32, tag="bqt")
            bvb = const.tile([128, H], f32, tag="bvb")
            ones = const.tile([128, 2], f32r, tag="ones")
            nc.sync.dma_start(out=bkt[:], in_=Bkd[:])
            nc.sync.dma_start(out=bqt[:], in_=Bqd[:])
            nc.sync.dma_start(out=bvb[:], in_=Bvd[:])
            nc.sync.dma_start(out=ones[:], in_=ONESd[:])

            for b in range(BPC):
                # ---- load activations (transposed: [d, t]) ----
                xt = data.tile([128, DC, T], f32r, tag="xt")
                yt = data.tile([128, DC, T], f32r, tag="yt")
                for c in range(DC):
                    nc.sync.dma_start(out=xt[:, c], in_=XTd[b, c])
                    nc.sync.dma_start(out=yt[:, c], in_=YTd[b, c])

                # ---- projections ----
                # KT[h, s] / QT[h, t]: lhsT = W[d_chunk, h_tile], rhs = XT/YT
                kt = data.tile([128, HC, T], f32r, tag="kt")
                qt = data.tile([128, HC, T], f32r, tag="qt")
                for w, src, dst, bias in (
                    (wk, xt, kt, bkt),
                    (wq, yt, qt, bqt),
                ):
                    for j in range(HC):
                        for hf in range(2):
                            ps = psum.tile([128, 512], f32, tag="proj")
                            for c in range(DC):
                                nc.tensor.matmul(
                                    ps[:],
                                    w[:, c, j * 128:(j + 1) * 128],
                                    src[:, c, hf * 512:(hf + 1) * 512],
                                    start=(c == 0), stop=(c == DC - 1),
                                )
                            nc.scalar.activation(
                                dst[:, j, hf * 512:(hf + 1) * 512], ps[:],
                                IDENT, bias=bias[:, j:j + 1], scale=1.0,
                            )
                # V[s, h] (normal layout): lhsT = XT[d_chunk, s_tile], rhs = Wv
                v = data.tile([128, TC, H], f32r, tag="v")
                for i in range(TC):
                    ps = psum.tile([128, 512], f32, tag="proj")
                    for c in range(DC):
                        nc.tensor.matmul(
                            ps[:],
                            xt[:, c, i * 128:(i + 1) * 128],
                            wv[:, c],
                            start=(c == 0), stop=(c == DC - 1),
                        )
                    nc.vector.tensor_add(v[:, i], ps[:], bvb[:])

                # ---- transposed masked-exp scores ----
                # ST[s, t] = K @ Q^T; PT = dag * exp(ST/sqrt(H))
                pt = data.tile([128, TC, T], f32r, tag="pt")
                for i in range(TC):
                    for hf in range(2):
                        ps = psum.tile([128, 512], f32, tag="st")
                        for j in range(HC):
                            nc.tensor.matmul(
                                ps[:],
                                kt[:, j, i * 128:(i + 1) * 128],
                                qt[:, j, hf * 512:(hf + 1) * 512],
                                start=(j == 0), stop=(j == HC - 1),
                            )
                        tmp = small.tile([128, 512], f32, tag="exp")
                        nc.scalar.activation(tmp[:], ps[:], EXP,
                                             bias=0.0, scale=SCALE)
                        nc.vector.tensor_mul(
                            pt[:, i, hf * 512:(hf + 1) * 512], tmp[:],
                            dag[:, i, hf * 512:(hf + 1) * 512],
                        )

                # ---- U = PT^T @ V, l = PT^T @ 1, O = U / l ----
                for t_ in range(TC):
                    up = psum.tile([128, 512], f32, tag="u")
                    lp = psum.tile([128, 2], f32, tag="l")
                    for i in range(TC):
                        lhsT = pt[:, i, t_ * 128:(t_ + 1) * 128]
                        nc.tensor.matmul(up[:], lhsT, v[:, i],
                                         start=(i == 0), stop=(i == TC - 1))
                        nc.tensor.matmul(lp[:], lhsT, ones[:],
                                         start=(i == 0), stop=(i == TC - 1))
                    lsb = small.tile([128, 2], f32, tag="lsb")
                    nc.vector.tensor_scalar_max(lsb[:], lp[:], 1e-30)
                    linv = small.tile([128, 2], f32, tag="linv")
                    nc.vector.reciprocal(linv[:], lsb[:])
                    osb = small.tile([128, 512], f32, tag="osb")
                    nc.vector.tensor_scalar_mul(osb[:], up[:], linv[:, 0:1])
                    nc.sync.dma_start(out=Od[b, t_ * 128:(t_ + 1) * 128],
                                      in_=osb[:])

    nc.compile()
    return nc


def _get_nc():
    global _CACHED_NC
    if _CACHED_NC is None:
        _CACHED_NC = _build()
    return _CACHED_NC


def _prep_core_inputs(X, Y, dag, Wk, bk, Wq, bq, Wv, bv):
    """Build the 8 per-core input maps (host-side shard + transpose)."""
    X = np.ascontiguousarray(np.asarray(X, dtype=np.float32))
    Y = np.ascontiguousarray(np.asarray(Y, dtype=np.float32))
    dag = np.ascontiguousarray(np.asarray(dag, dtype=np.float32))
    dag_r = dag.reshape(TC, 128, T)
    shared = {
        "dagr": dag_r,
        "Wkr": np.asarray(Wk, np.float32).reshape(DC, 128, H),
        "Wqr": np.asarray(Wq, np.float32).reshape(DC, 128, H),
        "Wvr": np.asarray(Wv, np.float32).reshape(DC, 128, H),
        "bkt": np.ascontiguousarray(
            np.asarray(bk, np.float32).reshape(HC, 128).T),
        "bqt": np.ascontiguousarray(
            np.asarray(bq, np.float32).reshape(HC, 128).T),
        "bvb": np.ascontiguousarray(
            np.broadcast_to(np.asarray(bv, np.float32), (128, H))),
        "ones": np.ones((128, 2), dtype=np.float32),
    }
    in_maps = []
    for core in range(NCORES):
        sl = slice(core * BPC, (core + 1) * BPC)
        xt = np.ascontiguousarray(X[sl].transpose(0, 2, 1)).reshape(
            BPC, DC, 128, T)
        yt = np.ascontiguousarray(Y[sl].transpose(0, 2, 1)).reshape(
            BPC, DC, 128, T)
        in_maps.append({"XT": xt, "YT": yt, **shared})
    return in_maps


def kernel(X, Y, dag, Wk, bk, Wq, bq, Wv, bv):
    nc = _get_nc()
    in_maps = _prep_core_inputs(X, Y, dag, Wk, bk, Wq, bq, Wv, bv)
    res = run_bass_kernel_spmd(nc, in_maps, list(range(NCORES)))
    return np.concatenate([res.results[i]["O"] for i in range(NCORES)],
                          axis=0)


# revision 18
# speedup vs baseline: 14101.5107x; 14101.5107x over previous
"""Sparse (DAG-masked) attention head on 8 Trainium2 NeuronCores.

Reference computation (per batch b of 64):
    K = X_b @ Wk + bk; Q = Y_b @ Wq + bq; V = X_b @ Wv + bv         [T=1024, H=512]
    S = Q @ K^T / sqrt(H); A = softmax(where(dag.T*S == 0, -inf, dag.T*S))
    O = A @ V   (fully-masked rows -> 0)

Strategy: data-parallel over batch (8 batches per core); weights + dag
replicated. All matmuls run in float32r (TF32-like, 1 cycle/row on PE,
~1e-4 relative error).

Key algebraic fusion: softmax over s is invariant to additive terms that
vary only in t, so
    S^T[s,t] = (X G Y^T)[s,t] + beta[s] + (t-only terms, cancel)
with G = Wk @ Wq^T and beta = X @ (Wk @ bq), both folded on the host.
This removes one of the three projections and both K/Q bias adds.

Scores are computed TRANSPOSED (ST[s,t] = Z @ Y^T with Z^T = G^T X^T) so
the softmax weights PT = dag * exp(ST/sqrt(H) + beta*scale) land directly
in the [s, t] layout needed as the stationary operand of the P @ V
matmul -- no on-chip transposes. Softmax skips max-subtraction (scores
are ~N(0,1); exp cannot overflow) and normalizes AFTER the V-matmul:
l[t] = sum_s PT is computed as a row via a ones-stationary matmul, then
scattered to per-partition layout by tiny SBUF->SBUF DMAs.

Host-side prep: X/Y are transposed to [D, T] per batch (the PE contracts
over the partition dim).
"""

import numpy as np

import concourse.bass as bass
import concourse.mybir as mybir
import concourse.tile as tile
from concourse import bacc
from concourse.bass_utils import run_bass_kernel_spmd

B, T, D, H = 64, 1024, 512, 512
NCORES = 8
BPC = B // NCORES          # batches per core
DC = D // 128              # d chunks (4)
TC = T // 128              # t/s tiles (8)
SCALE = 1.0 / float(np.sqrt(H))

f32 = mybir.dt.float32
f32r = mybir.dt.float32r
bf16 = mybir.dt.bfloat16
f16 = mybir.dt.float16
EXP = mybir.ActivationFunctionType.Exp
COPY = mybir.ActivationFunctionType.Copy

_CACHED_NC = None

MM_DT = f32r               # matmul operand dtype: f32r (accurate) or bf16


def _build(reps=1, mm_dt=None):
    # reps>1 wraps the whole pipeline in a hardware loop that re-runs it on
    # the same data -- used only by the timing harness (wall-clock deltas
    # cancel the axon RPC overhead).
    dt = MM_DT if mm_dt is None else mm_dt
    nc = bacc.Bacc("TRN2", target_bir_lowering=False, debug=False,
                   num_devices=NCORES)

    XTd = nc.dram_tensor("XT", [BPC, DC, 128, T], dt, kind="ExternalInput").ap()
    YTd = nc.dram_tensor("YT", [BPC, DC, 128, T], dt, kind="ExternalInput").ap()
    DAGd = nc.dram_tensor("dagr", [TC, 128, T], bf16, kind="ExternalInput").ap()
    Gd = nc.dram_tensor("Gr", [DC, 128, D], dt, kind="ExternalInput").ap()
    gd = nc.dram_tensor("gr", [DC, 128, 2], dt, kind="ExternalInput").ap()
    Wvd = nc.dram_tensor("Wvr", [DC, 128, H], dt, kind="ExternalInput").ap()
    Bvd = nc.dram_tensor("bvb", [128, H], f32, kind="ExternalInput").ap()
    ONESd = nc.dram_tensor("ones", [128, 2], dt, kind="ExternalInput").ap()
    Od = nc.dram_tensor("O", [BPC, T, H], f32, kind="ExternalOutput").ap()

    with tile.TileContext(nc) as tc:
        with (
            tc.tile_pool(name="const", bufs=1) as const,
            tc.tile_pool(name="data", bufs=1) as data,
            tc.tile_pool(name="data2", bufs=2) as data2,
            tc.tile_pool(name="pipe", bufs=2) as pipe,
            tc.tile_pool(name="small", bufs=3) as small,
            tc.tile_pool(name="psum3", bufs=3, space="PSUM") as psum3,
            tc.tile_pool(name="psum1", bufs=1, space="PSUM") as psum1,
        ):
            # ---- resident tensors ----
            # Input streams split across the three DMA-capable queues
            # (SP / ACT / GPSIMD); batch-0 activations interleaved with G in
            # consumption order so the first matmul starts after ~1MB of DMA.
            gt = const.tile([128, DC, D], dt, tag="gt")
            gv = const.tile([128, DC, 2], dt, tag="gv")
            wv = const.tile([128, DC, H], dt, tag="wv")
            bvb = const.tile([128, H], f32, tag="bvb")
            ones = const.tile([128, 2], dt, tag="ones")
            dag = const.tile([128, TC, T], bf16, tag="dag")
            if reps == 1:
                xt0 = data2.tile([128, DC, T], dt, tag="xt")
                yt0 = data.tile([128, DC, T], dt, tag="yt")
                for c in range(DC):
                    nc.sync.dma_start(out=gt[:, c], in_=Gd[c])
                    nc.sync.dma_start(out=xt0[:, c], in_=XTd[0, c])
                for c in range(DC):
                    nc.scalar.dma_start(out=yt0[:, c], in_=YTd[0, c])
            else:
                xt0 = yt0 = None
                for c in range(DC):
                    nc.sync.dma_start(out=gt[:, c], in_=Gd[c])
            for c in range(DC):
                nc.gpsimd.dma_start(out=wv[:, c], in_=Wvd[c])
                nc.gpsimd.dma_start(out=gv[:, c], in_=gd[c])
            nc.gpsimd.dma_start(out=bvb[:], in_=Bvd[:])
            nc.gpsimd.dma_start(out=ones[:], in_=ONESd[:])
            for i in range(TC):
                nc.gpsimd.dma_start(out=dag[:, i], in_=DAGd[i])

            def emit_batch(b):
                # ---- load activations (transposed: [d, t]) ----
                # xt (used by ZT/V/beta, early) on SP, double-buffered;
                # yt (used by scores, later) on ACT.
                if b == 0 and xt0 is not None:
                    xt, yt = xt0, yt0
                else:
                    xt = data2.tile([128, DC, T], dt, tag="xt")
                    yt = data.tile([128, DC, T], dt, tag="yt")
                    for c in range(DC):
                        nc.sync.dma_start(out=xt[:, c], in_=XTd[b, c])
                        nc.scalar.dma_start(out=yt[:, c], in_=YTd[b, c])

                # ---- ZT[d', s] = G^T X^T: lhsT = G[d, d'_tile], rhs = XT ----
                zt = data.tile([128, DC, T], dt, tag="zt")
                for j in range(DC):
                    for hf in range(2):
                        ps = psum3.tile([128, 512], f32, tag="proj")
                        for c in range(DC):
                            nc.tensor.matmul(
                                ps[:],
                                gt[:, c, j * 128:(j + 1) * 128],
                                xt[:, c, hf * 512:(hf + 1) * 512],
                                start=(c == 0), stop=(c == DC - 1),
                            )
                        nc.scalar.activation(
                            zt[:, j, hf * 512:(hf + 1) * 512], ps[:],
                            COPY, bias=0.0, scale=1.0,
                        )
                # ---- V[s, h]: lhsT = XT[d, s_tile], rhs = Wv ----
                v = data.tile([128, TC, H], dt, tag="v")
                for i in range(TC):
                    ps = psum3.tile([128, 512], f32, tag="proj")
                    for c in range(DC):
                        nc.tensor.matmul(
                            ps[:],
                            xt[:, c, i * 128:(i + 1) * 128],
                            wv[:, c],
                            start=(c == 0), stop=(c == DC - 1),
                        )
                    nc.vector.tensor_add(v[:, i], ps[:], bvb[:])

                # ---- beta[s] = scale * X @ (Wk bq), as a row, scattered to
                # per-partition [128, TC] for use as the exp bias.
                beta_row = small.tile([2, T], f32, tag="brow")
                for hf in range(2):
                    ps = psum1.tile([2, 512], f32, tag="l")
                    for c in range(DC):
                        nc.tensor.matmul(
                            ps[:], gv[:, c],
                            xt[:, c, hf * 512:(hf + 1) * 512],
                            start=(c == 0), stop=(c == DC - 1),
                        )
                    nc.vector.tensor_scalar_mul(
                        beta_row[:, hf * 512:(hf + 1) * 512], ps[:], SCALE)
                beta = small.tile([128, TC], f32, tag="beta")
                for i in range(TC):
                    nc.sync.dma_start(
                        out=beta[:, i:i + 1],
                        in_=beta_row[0:1, i * 128:(i + 1) * 128])

                # ---- scores + AV in two t-halves (AV of one half overlaps
                # the score matmuls of the next).
                for th in range(2):
                    t0 = th * 512
                    # PT[s, t] = dag * exp(ST*scale + beta), ST = Z @ Y^T
                    pt = pipe.tile([128, TC, 512], dt, tag="pt")
                    for i in range(TC):
                        ps = psum3.tile([128, 512], f32, tag="st")
                        for j in range(DC):
                            nc.tensor.matmul(
                                ps[:],
                                zt[:, j, i * 128:(i + 1) * 128],
                                yt[:, j, t0:t0 + 512],
                                start=(j == 0), stop=(j == DC - 1),
                            )
                        tmp = small.tile([128, 512], f32, tag="exp")
                        nc.scalar.activation(tmp[:], ps[:], EXP,
                                             bias=beta[:, i:i + 1],
                                             scale=SCALE)
                        nc.vector.tensor_mul(
                            pt[:, i], tmp[:], dag[:, i, t0:t0 + 512],
                        )

                    # l as a row via ones-stationary matmul, scattered to
                    # per-partition [128, 4]; ready before the U groups end.
                    lp = psum1.tile([2, 512], f32, tag="l")
                    for i in range(TC):
                        nc.tensor.matmul(lp[:], ones[:], pt[:, i],
                                         start=(i == 0), stop=(i == TC - 1))
                    lrow = small.tile([2, 512], f32, tag="lrow")
                    nc.vector.tensor_scalar_max(lrow[:], lp[:], 1e-30)
                    lcol = small.tile([128, 4], f32, tag="lcol")
                    for tq in range(4):
                        nc.sync.dma_start(
                            out=lcol[:, tq:tq + 1],
                            in_=lrow[0:1, tq * 128:(tq + 1) * 128])
                    linv = small.tile([128, 4], f32, tag="linv")
                    nc.vector.reciprocal(linv[:], lcol[:])

                    # U = PT^T @ V; O = U / l
                    for tq in range(4):
                        t_ = th * 4 + tq
                        up = psum1.tile([128, 512], f32, tag="u")
                        for i in range(TC):
                            lhsT = pt[:, i, tq * 128:(tq + 1) * 128]
                            nc.tensor.matmul(up[:], lhsT, v[:, i],
                                             start=(i == 0),
                                             stop=(i == TC - 1))
                        osb = small.tile([128, 512], f32, tag="osb")
                        nc.scalar.activation(osb[:], up[:], COPY,
                                             bias=0.0,
                                             scale=linv[:, tq:tq + 1])
                        nc.scalar.dma_start(
                            out=Od[b, t_ * 128:(t_ + 1) * 128], in_=osb[:])

            if reps == 1:
                for b in range(BPC):
                    emit_batch(b)
            else:
                with tc.For_i(0, reps, 1):
                    for b in range(BPC):
                        emit_batch(b)

    nc.compile()
    return nc


def _get_nc():
    global _CACHED_NC
    if _CACHED_NC is None:
        _CACHED_NC = _build()
    return _CACHED_NC


def _prep_core_inputs(X, Y, dag, Wk, bk, Wq, bq, Wv, bv, mm_dt=None):
    """Build the 8 per-core input maps (host-side shard + transpose +
    weight fusion G = Wk Wq^T, g = Wk bq)."""
    import ml_dtypes
    dt = MM_DT if mm_dt is None else mm_dt
    mmnp = {bf16: ml_dtypes.bfloat16, f16: np.float16}.get(dt, np.float32)
    X = np.ascontiguousarray(np.asarray(X, dtype=np.float32))
    Y = np.ascontiguousarray(np.asarray(Y, dtype=np.float32))
    dag = np.ascontiguousarray(np.asarray(dag, dtype=np.float32))
    dag_r = dag.reshape(TC, 128, T).astype(ml_dtypes.bfloat16)
    Wk64 = np.asarray(Wk, np.float64)
    G = (Wk64 @ np.asarray(Wq, np.float64).T).astype(np.float32)
    g = (Wk64 @ np.asarray(bq, np.float64)).astype(np.float32)
    shared = {
        "dagr": dag_r,
        "Gr": G.reshape(DC, 128, D).astype(mmnp),
        "gr": np.repeat(g.reshape(DC, 128, 1), 2, axis=2).astype(mmnp),
        "Wvr": np.asarray(Wv, np.float32).reshape(DC, 128, H).astype(mmnp),
        "bvb": np.ascontiguousarray(
            np.broadcast_to(np.asarray(bv, np.float32), (128, H))),
        "ones": np.ones((128, 2), dtype=mmnp),
    }
    in_maps = []
    for core in range(NCORES):
        sl = slice(core * BPC, (core + 1) * BPC)
        xt = np.ascontiguousarray(X[sl].transpose(0, 2, 1)).reshape(
            BPC, DC, 128, T).astype(mmnp, copy=False)
        yt = np.ascontiguousarray(Y[sl].transpose(0, 2, 1)).reshape(
            BPC, DC, 128, T).astype(mmnp, copy=False)
        in_maps.append({"XT": xt, "YT": yt, **shared})
    return in_maps


def kernel(X, Y, dag, Wk, bk, Wq, bq, Wv, bv):
    nc = _get_nc()
    in_maps = _prep_core_inputs(X, Y, dag, Wk, bk, Wq, bq, Wv, bv)
    last_err = None
    for _attempt in range(3):
        try:
            res = run_bass_kernel_spmd(nc, in_maps, list(range(NCORES)))
            break
        except Exception as e:  # transient NRT device errors -- retry
            last_err = e
    else:
        raise last_err
    return np.concatenate([res.results[i]["O"] for i in range(NCORES)],
                          axis=0)
